# revision 21
# baseline (speedup 1.0000x reference)
"""Depthwise morphological (max-plus) dilation, 3x3, stride 1, zero-pad 1.

out[b,c,i,j] = max_{p,q} ( x_pad[b,c,i+p,j+q] + se[c,p,q] )

Sharding: pure data parallel over batch (16 batches -> 8 cores x 2).
On-core layout: partition dim = 2 batches x 64 channels = 128 planes;
each partition processes its own plane in row-blocks. The host supplies
x zero-padded to [P, H+2, W+2] (and fp16-converted in f16 mode), so all
9 taps are free-dim-shifted 3D access patterns of one SBUF tile and the
device does no zero-fill at all.
"""

import numpy as np

B, C, H, W = 16, 64, 256, 256
K = 3
NCORES = 8
BPC = B // NCORES          # batches per core
P = BPC * C                # 128 partitions
HP, WP = H + 2, W + 2      # host-padded plane

COMPUTE = "f16"            # "f16" (fast, ~5e-4 rel err) or "f32" (exact)
R = 32                     # output rows per block

_prog_cache = {}


def _build(compute=COMPUTE, h=H, r=R, gpsimd_taps=(), reps=1):
    """Build the Bass program for one core: x [P,h+2,W+2] -> o [P,h,W]."""
    import concourse.bacc as bacc
    import concourse.mybir as mybir
    from concourse.tile import TileContext

    add, mx = mybir.AluOpType.add, mybir.AluOpType.max
    dt = mybir.dt.float16 if compute == "f16" else mybir.dt.float32

    nc = bacc.Bacc()
    x_d = nc.dram_tensor("x", [P, h + 2, W + 2], dt, kind="ExternalInput")
    se_d = nc.dram_tensor("se", [P, K * K], mybir.dt.float32, kind="ExternalInput")
    o_d = nc.dram_tensor("o", [P, h, W], dt, kind="ExternalOutput")

    with TileContext(nc) as tc:
        with (
            tc.tile_pool(name="cpool", bufs=1) as cpool,
            tc.tile_pool(name="xpool", bufs=4) as xpool,
            tc.tile_pool(name="apool", bufs=4) as apool,
            tc.tile_pool(name="tpool", bufs=2) as tpool,
            tc.tile_pool(name="abspool", bufs=1) as abspool,
        ):
            se_sb = cpool.tile([P, K * K], mybir.dt.float32)
            scratch = cpool.tile([P, 2], mybir.dt.float32)
            nc.sync.dma_start(out=se_sb[:], in_=se_d[:, :])
            # tiny DVE read of se_sb: absorbs its DMA-sem wait so compute
            # ops never carry >1 sync wait (TS descriptor limit).
            nc.vector.tensor_copy(scratch[:, 0:2], se_sb[:, 0:2])

            n_iters = h // r
            for r0 in [v for _ in range(reps) for v in range(0, h, r)]:
                xt = xpool.tile([P, r + 2, W + 2], dt, tag="xt")
                nc.sync.dma_start(out=xt[:], in_=x_d[:, r0 : r0 + r + 2, :])

                # DMA-wait absorber: a tiny DVE read of the fresh x tile into
                # a never-recycled slot (bufs=n_iters), so it carries ONLY the
                # x-DMA wait. Every compute op then carries <=1 sync wait
                # (TS descriptor limit is 1).
                absorb = abspool.tile([P, 1, 2], dt, tag="ab",
                                      bufs=min(n_iters * reps, 16))
                nc.vector.tensor_copy(absorb[:], xt[:, 0:1, 0:2])

                acc = apool.tile([P, r, W], dt, tag="acc")
                taps = [(di, dj) for di in range(K) for dj in range(K)]
                # tap 0 in two steps: the TS takes the tmp-slot wait, the
                # copy takes the acc-slot-release (output DMA) wait.
                di, dj = taps[0]
                tmp = tpool.tile([P, r, W], dt, tag="tmp")
                nc.vector.tensor_scalar(
                    tmp[:], xt[:, di : di + r, dj : dj + W],
                    se_sb[:, 0:1], None, add,
                )
                nc.vector.tensor_copy(acc[:], tmp[:])
                for t, (di, dj) in enumerate(taps[1:], start=1):
                    src = xt[:, di : di + r, dj : dj + W]
                    s = se_sb[:, t : t + 1]
                    if t in gpsimd_taps:
                        nc.gpsimd.scalar_tensor_tensor(acc[:], src, s, acc[:], add, mx)
                    elif compute == "f16":
                        tmp = tpool.tile([P, r, W], dt, tag="tmp")
                        nc.vector.tensor_scalar(tmp[:], src, s, None, add)
                        nc.vector.tensor_tensor(acc[:], acc[:], tmp[:], mx)
                    else:
                        nc.vector.scalar_tensor_tensor(acc[:], src, s, acc[:], add, mx)

                nc.sync.dma_start(out=o_d[:, r0 : r0 + r, :], in_=acc[:])
    # bacc legalization (splits >1-wait instructions into event semaphores)
    nc.finalize()
    return nc


def _get_prog(key=("default",)):
    if key not in _prog_cache:
        _prog_cache[key] = _build()
    return _prog_cache[key]


def _pad_shard(x_shard, np_dt):
    """[BPC,C,H,W] fp32 -> zero-padded [P, H+2, W+2] in np_dt."""
    xp = np.zeros((P, HP, WP), np_dt)
    xp[:, 1 : H + 1, 1 : W + 1] = x_shard.reshape(P, H, W)
    return xp


def _run(x, se, **spmd_kwargs):
    from concourse.bass_utils import run_bass_kernel_spmd

    nc = _get_prog()
    np_dt = np.float16 if COMPUTE == "f16" else np.float32
    x = np.asarray(x)
    se_p = np.tile(np.asarray(se, np.float32).reshape(C, K * K), (BPC, 1))
    in_maps = [
        {"x": _pad_shard(x[k * BPC : (k + 1) * BPC], np_dt), "se": se_p}
        for k in range(NCORES)
    ]
    res = run_bass_kernel_spmd(nc, in_maps, core_ids=list(range(NCORES)), **spmd_kwargs)
    out = np.empty((B, C, H, W), np.float32)
    for k in range(NCORES):
        out[k * BPC : (k + 1) * BPC] = (
            res.results[k]["o"].astype(np.float32).reshape(BPC, C, H, W)
        )
    return out, res


def kernel(x: np.ndarray, se: np.ndarray) -> np.ndarray:
    return _run(x, se)[0]
